# revision 14
# baseline (speedup 1.0000x reference)
"""Trainium2 Bass kernel for nn_DMHA_3255585210402 (retrieval_knn DMHA).

Key algebraic fact: TOPK == NVK == 4, so jax.lax.top_k over the size-4 v_keys
axis selects *all* entries; the gather+sum over (DVH, TOPK) therefore reduces
to a constant vector c = 2 * v_embed[0:4].sum(0), and the whole
compute_value_states branch collapses to  v = x * c.

So the module is a causal MHA layer (B=2, H=16, T=2048, HD=128, D=2048) with
elementwise-scaled V.  Sharding: 8 cores = 2 batches x 4 head-groups.

All matmuls run in bf16 (same 1 cycle/row as fp32r on the PE but FWL weight
loads kick in, and ACT/DVE/SBUF byte traffic halves).  fp8 was measured and
rejected: softmax-logit noise from fp8 q/k lands directly on the output
(rel err 4.5e-2 > 2e-2 gate) because V is random -- no averaging rescue.

Schedule highlights:
  * phase A (q/k projections) streams x dk-inner, one psum group per
    (weight, head); drains pipeline on ACT.  The first query chunk of
    attention is woven between the last three x-chunks of phase A so the
    exp latency of those thin heads hides behind queued projection matmuls.
  * c is folded into the V operand on the host (xgc = x * c); V is never
    materialized.
  * exps are batched [128, 2, 512] across psum-bank pairs; column sums are
    built as a DVE pair/quad/oct tree and hit the PE as one ones-matmul per
    8 key chunks with a full 128-wide stationary, so every psum row carries
    the colsum and the normalize is reciprocal+multiply on the DVE -- no
    partition broadcast.
  * scores/o matmuls and the causal mask are trimmed to live columns of
    diagonal tiles.
  * out-proj (phase C) rides behind the attention head loop one query chunk
    back on its own psum banks; its drains run on ACT; output is bf16.
"""

import math

import numpy as np
import ml_dtypes

import concourse.bass as bass
import concourse.mybir as mybir
import concourse.tile as tile
from concourse import bacc
from concourse.bass_utils import run_bass_kernel_spmd

B, T, D = 2, 2048, 2048
H, HD = 16, 128
G = 4              # head-groups (cores per batch)
GH = H // G        # heads per core
GF = GH * HD       # projected features per core (512)
NCORES = 8
P = 128            # partitions
TQ = 512           # tq chunk width (psum bank / fp32 moving max)
F32 = mybir.dt.float32
BF16 = mybir.dt.bfloat16

DK = D // P        # 16 contraction chunks for projections
NTQ = T // TQ      # 4 query chunks
NTK = T // P       # 16 key chunks


def _body(tc, xT, xgc, wqT, wkT, woT, bqT, bkT, out):
    nc = tc.nc
    sc_exp = 1.0 / math.sqrt(HD)

    with (
        tc.tile_pool(name="const", bufs=1) as const,
        tc.tile_pool(name="res1", bufs=1) as res1,
        tc.tile_pool(name="res2", bufs=1) as res2,
        tc.tile_pool(name="wt", bufs=4) as wtp,
        tc.tile_pool(name="pr", bufs=6) as prp,
        tc.tile_pool(name="rb", bufs=4) as rbp,
        tc.tile_pool(name="ct", bufs=4) as ctp,
        tc.tile_pool(name="psO", bufs=1, space="PSUM") as psO,
        tc.tile_pool(name="psSum", bufs=1, space="PSUM") as psSum,
    ):
        qT_sb = res1.tile([P, GH, T], BF16)   # q, transposed per head
        kT_sb = res1.tile([P, GH, T], BF16)

        bq_sb = const.tile([HD, GH], F32)
        nc.sync.dma_start(out=bq_sb, in_=bqT)
        bk_sb = const.tile([HD, GH], F32)
        nc.sync.dma_start(out=bk_sb, in_=bkT)

        xg_sb = res2.tile([P, NTK, GF], BF16)  # (x*c)[:, gsl] by tk chunk
        oT_sb = res2.tile([P, GH, T], BF16)    # attention out, transposed
        wo_sb = res2.tile([P, GH, D], BF16)    # Wo[:, gsl].T chunked
        ones_sb = res2.tile([P, P], BF16)
        nc.vector.memset(ones_sb, 1.0)

        def emit_head(j, h, psS_pool, afill=None, nfill=0):
            """One attention head for query chunk j.  PE work that depends
            on an exp is lagged one s-pair; when `afill` is given, phase-A
            matmul groups are pulled in after each exp as extra cover."""
            qsl = slice(j * TQ, (j + 1) * TQ)
            nkk = (j + 1) * (TQ // P)
            npair = nkk // 2
            ncs = (j + 1) // 2 + (j + 1) % 2
            ps_o = psO.tile([P, TQ], F32, name="ps_o")
            ps_sum = psSum.tile([P, TQ], F32, name="ps_sum")
            wps = []
            quads = []
            ics = 0
            deferred = []
            for ip in range(npair):
                ps2 = psS_pool.tile([P, 2, TQ], F32, name="ps_s", tag="ps_s")
                for jj in (0, 1):
                    i = 2 * ip + jj
                    g = i - (TQ // P) * j
                    lo = g * P if g >= 2 else 0
                    nc.tensor.matmul(
                        ps2[:, jj, lo:],
                        kT_sb[:, h, i * P : (i + 1) * P],
                        qT_sb[:, h, j * TQ + lo : (j + 1) * TQ],
                        start=True,
                        stop=True,
                    )
                wt2 = wtp.tile([P, 2, TQ], BF16, name="wt")
                nc.scalar.activation(
                    wt2, ps2, mybir.ActivationFunctionType.Exp, scale=sc_exp,
                )
                for jj in (0, 1):
                    i = 2 * ip + jj
                    g = i - (TQ // P) * j
                    if g >= 0:  # zero cols left of + on the diagonal
                        nc.gpsimd.affine_select(
                            out=wt2[:, jj, 0 : (g + 1) * P],
                            in_=wt2[:, jj, 0 : (g + 1) * P],
                            pattern=[[1, (g + 1) * P]],
                            compare_op=mybir.AluOpType.is_ge,
                            fill=0.0,
                            base=-(P * g),
                            channel_multiplier=-1,
                        )
                for fn in deferred:
                    fn()
                deferred = []
                if afill is not None:
                    for _ in range(nfill):
                        grp = next(afill, None)
                        if grp is not None:
                            grp()
                wp = prp.tile([P, TQ], BF16, name="wp")
                nc.vector.tensor_add(wp, wt2[:, 0, :], wt2[:, 1, :])
                wps.append(wp)

                def _mk_o(ipc=ip, wt2c=wt2):
                    def emit():
                        for jj in (0, 1):
                            i = 2 * ipc + jj
                            g = i - (TQ // P) * j
                            lo = g * P if g >= 2 else 0
                            nc.tensor.matmul(
                                ps_o[:, lo:],
                                xg_sb[:, i, h * HD : (h + 1) * HD],
                                wt2c[:, jj, lo:],
                                start=(i == 0),
                                stop=(i == nkk - 1),
                            )
                    return emit

                deferred.append(_mk_o())
                if ip % 2 == 1:
                    wq4 = prp.tile([P, TQ], BF16, name="wq4")
                    nc.vector.tensor_add(wq4, wps[-2], wps[-1])
                    quads.append(wq4)
                    emit_cs = None
                    if len(quads) == 2:
                        oc = prp.tile([P, TQ], BF16, name="oc")
                        nc.vector.tensor_add(oc, quads[0], quads[1])
                        quads = []
                        emit_cs = oc
                    elif ip == npair - 1:
                        emit_cs = quads[0]
                        quads = []
                    if emit_cs is not None:
                        def _mk_cs(icsc=ics, rhs=emit_cs):
                            def emit():
                                nc.tensor.matmul(
                                    ps_sum, ones_sb, rhs,
                                    start=(icsc == 0),
                                    stop=(icsc == ncs - 1),
                                )
                            return emit

                        deferred.append(_mk_cs())
                        ics += 1
            for fn in deferred:
                fn()
            # normalize immediately: recip of the row-broadcast colsums,
            # then scale ps_o on the DVE.  The PE moves on with no
            # dependency on this chain.
            rb = rbp.tile([P, TQ], F32, name="rb")
            nc.vector.reciprocal_approx_fast(out=rb, in_=ps_sum)
            nc.vector.tensor_mul(oT_sb[:, h, qsl], ps_o, rb)

        # --- phase A: q/k projections (bf16, dk-inner), with the j=0
        # attention chunk woven between its last three x-chunks ---
        with (
            tc.tile_pool(name="wqk", bufs=1) as wqk,
            tc.tile_pool(name="xt", bufs=2) as xtp,
            tc.tile_pool(name="psA", bufs=2, space="PSUM") as psA,
            tc.tile_pool(name="psSw", bufs=2, space="PSUM") as psSw,
        ):
            wq_sb = wqk.tile([P, DK, GF], BF16)
            wk_sb = wqk.tile([P, DK, GF], BF16)
            # chunk the first DMAs so the first matmul (dk=0) starts as
            # soon as the first slices land; wk is only needed after the
            # whole q pass, so its chunks go last
            xt0 = xtp.tile([P, DK, TQ], BF16, name="xt")
            slices = [slice(d, d + 1) for d in range(DK)]
            for dsl in slices:
                nc.sync.dma_start(out=wq_sb[:, dsl, :], in_=wqT[:, dsl, :])
                nc.sync.dma_start(out=xt0[:, dsl, :], in_=xT[0][:, dsl, :])
            # gpsimd library (affine_select) fetch rides behind the
            # first-matmul-critical chunks; first use is much later
            from concourse import library_config
            nc.gpsimd.load_library(library_config.attn)
            for dsl in slices:
                nc.sync.dma_start(out=wk_sb[:, dsl, :], in_=wkT[:, dsl, :])
            nc.sync.dma_start(out=xg_sb, in_=xgc)
            nc.sync.dma_start(out=wo_sb, in_=woT)

            def _mk_agroup(xt, tsl, w_sb, dstT, bias, h, on_dve=False):
                def emit():
                    ps = psA.tile([P, TQ], F32, name="psA_t")
                    for dk in range(DK):
                        nc.tensor.matmul(
                            ps,
                            w_sb[:, dk, h * HD : (h + 1) * HD],
                            xt[:, dk, :],
                            start=(dk == 0),
                            stop=(dk == DK - 1),
                        )
                    if on_dve:
                        # interleaved groups drain on the DVE: the ACT queue
                        # is busy with the j=0 exps there
                        nc.vector.tensor_scalar(
                            dstT[:, h, tsl], ps, bias[:, h : h + 1], None,
                            mybir.AluOpType.add,
                        )
                    else:
                        nc.scalar.activation(
                            dstT[:, h, tsl],
                            ps,
                            mybir.ActivationFunctionType.Identity,
                            bias=bias[:, h : h + 1],
                        )
                return emit

            agroups = []
            for tci in range(NTQ):
                tsl = slice(tci * TQ, (tci + 1) * TQ)
                if tci == 0:
                    xt = xt0
                else:
                    xt = xtp.tile([P, DK, TQ], BF16, name="xt")
                    nc.sync.dma_start(out=xt, in_=xT[tci])
                for w_sb, dstT, bias in (
                    (wq_sb, qT_sb, bq_sb),
                    (wk_sb, kT_sb, bk_sb),
                ):
                    for h in range(GH):
                        agroups.append(
                            _mk_agroup(xt, tsl, w_sb, dstT, bias, h,
                                       on_dve=(tci > 0))
                        )

            for grp in agroups[: 2 * GH]:  # all of tci=0
                grp()
            afill = iter(agroups[2 * GH :])  # 24 groups left
            for h in range(GH):
                emit_head(0, h, psSw, afill=afill, nfill=3)
            for grp in afill:
                grp()

        # --- query chunks 1..3 with out-proj riding one chunk behind ---
        with (
            tc.tile_pool(name="psS", bufs=2, space="PSUM") as psS,
            tc.tile_pool(name="psC", bufs=2, space="PSUM") as psC,
        ):
            for j in range(1, NTQ):
                for h in range(GH):
                    emit_head(j, h, psS)
                    _emit_outproj(nc, psC, ctp, wo_sb, oT_sb, out,
                                  j - 1, (4 * h, 4 * h + 4))
            _emit_outproj(nc, psC, ctp, wo_sb, oT_sb, out, NTQ - 1, (0, DK))


def _emit_outproj(nc, psC, ctp, wo_sb, oT_sb, out, j, dk_range):
    qsl = slice(j * TQ, (j + 1) * TQ)
    for dk in range(*dk_range):
        ps = psC.tile([P, TQ], F32, name="psC_t")
        for m in range(GH):
            nc.tensor.matmul(
                ps,
                wo_sb[:, m, dk * P : (dk + 1) * P],
                oT_sb[:, m, qsl],
                start=(m == 0),
                stop=(m == GH - 1),
            )
        ct = ctp.tile([P, TQ], BF16, name="ct")
        nc.scalar.copy(ct, ps)
        nc.sync.dma_start(out=out[:, dk, qsl], in_=ct)


def build_program():
    nc = bacc.Bacc(
        "TRN2", target_bir_lowering=False, debug=False, num_devices=NCORES
    )
    xT = nc.dram_tensor("xT", [NTQ, P, DK, TQ], BF16, kind="ExternalInput").ap()
    xgc = nc.dram_tensor("xgc", [P, NTK, GF], BF16, kind="ExternalInput").ap()
    wqT = nc.dram_tensor("wqT", [P, DK, GF], BF16, kind="ExternalInput").ap()
    wkT = nc.dram_tensor("wkT", [P, DK, GF], BF16, kind="ExternalInput").ap()
    woT = nc.dram_tensor("woT", [P, GH, D], BF16, kind="ExternalInput").ap()
    bqT = nc.dram_tensor("bqT", [HD, GH], F32, kind="ExternalInput").ap()
    bkT = nc.dram_tensor("bkT", [HD, GH], F32, kind="ExternalInput").ap()
    out = nc.dram_tensor("out", [P, DK, T], BF16, kind="ExternalOutput").ap()

    with tile.TileContext(nc) as tc:
        _body(tc, xT, xgc, wqT, wkT, woT, bqT, bkT, out)
    nc.compile()
    return nc


_NC_CACHE = None
LAST_RESULT = None
TRACE = False


def kernel(x, Wq, bq, Wk, bk, Wvq, bvq, v_keys, v_embed, Wo, bo):
    global _NC_CACHE, LAST_RESULT
    x = np.asarray(x, np.float32)
    Wq = np.asarray(Wq, np.float32)
    bq = np.asarray(bq, np.float32)
    Wk = np.asarray(Wk, np.float32)
    bk = np.asarray(bk, np.float32)
    v_embed = np.asarray(v_embed, np.float32)
    Wo = np.asarray(Wo, np.float32)
    bo = np.asarray(bo, np.float32)

    bf = ml_dtypes.bfloat16
    c = 2.0 * v_embed[:G].sum(axis=0)

    in_maps = []
    for core in range(NCORES):
        b, g = divmod(core, G)
        gsl = slice(g * GF, (g + 1) * GF)
        # x[b].T chunked [tci][p][dk][t]: elem = x[b][tci*TQ+t, dk*P+p]
        xTd = np.ascontiguousarray(
            x[b].reshape(NTQ, TQ, DK, P).transpose(0, 3, 2, 1)
        ).astype(bf)
        # (x*c) slice chunked [p][i][f]: elem = (x*c)[b][i*P+p, gsl.start+f]
        xgcd = np.ascontiguousarray(
            (x[b][:, gsl] * c[gsl]).reshape(NTK, P, GF).transpose(1, 0, 2)
        ).astype(bf)
        # Wq[gsl].T chunked [p][dk][f]: elem = Wq[gsl][f, dk*P+p]
        wqTd = np.ascontiguousarray(
            Wq[gsl, :].T.reshape(DK, P, GF).transpose(1, 0, 2)
        ).astype(bf)
        wkTd = np.ascontiguousarray(
            Wk[gsl, :].T.reshape(DK, P, GF).transpose(1, 0, 2)
        ).astype(bf)
        # Wo[:, gsl].T chunked [p][m][d]: elem = Wo[d, gsl.start + m*P+p]
        woTd = np.ascontiguousarray(
            Wo[:, gsl].T.reshape(GH, P, D).transpose(1, 0, 2)
        ).astype(bf)
        in_maps.append(
            {
                "xT": xTd,
                "xgc": xgcd,
                "wqT": wqTd,
                "wkT": wkTd,
                "woT": woTd,
                "bqT": np.ascontiguousarray(bq[gsl].reshape(GH, HD).T),
                "bkT": np.ascontiguousarray(bk[gsl].reshape(GH, HD).T),
            }
        )

    if _NC_CACHE is None:
        _NC_CACHE = build_program()
    res = run_bass_kernel_spmd(
        _NC_CACHE, in_maps, list(range(NCORES)), trace=TRACE
    )
    LAST_RESULT = res

    out = np.zeros((B, T, D), np.float32)
    for core in range(NCORES):
        b = core // G
        # out dram [p][dk][t]: elem = outT[dk*P+p, t] -> out[b][t, dk*P+p]
        o = res.results[core]["out"].astype(np.float32)
        out[b] += o.transpose(2, 1, 0).reshape(T, D)
    out += bo[None, None, :]
    return out


if __name__ == "__main__":
    nc = build_program()
    print("built ok")


# revision 15
# speedup vs baseline: 1.0359x; 1.0359x over previous
"""Trainium2 Bass kernel for nn_DMHA_3255585210402 (retrieval_knn DMHA).

Key algebraic fact: TOPK == NVK == 4, so jax.lax.top_k over the size-4 v_keys
axis selects *all* entries; the gather+sum over (DVH, TOPK) therefore reduces
to a constant vector c = 2 * v_embed[0:4].sum(0), and the whole
compute_value_states branch collapses to  v = x * c.

So the module is a causal MHA layer (B=2, H=16, T=2048, HD=128, D=2048) with
elementwise-scaled V.  Sharding: 8 cores = 2 batches x 4 head-groups.

All matmuls run in bf16 (same 1 cycle/row as fp32r on the PE but FWL weight
loads kick in, and ACT/DVE/SBUF byte traffic halves).  fp8 was measured and
rejected: softmax-logit noise from fp8 q/k lands directly on the output
(rel err 4.5e-2 > 2e-2 gate) because V is random -- no averaging rescue.

Schedule highlights:
  * phase A (q/k projections) streams x dk-inner, one psum group per
    (weight, head); drains pipeline on ACT.  The first query chunk of
    attention is woven between the last three x-chunks of phase A so the
    exp latency of those thin heads hides behind queued projection matmuls.
  * c is folded into the V operand on the host (xgc = x * c); V is never
    materialized.
  * exps are batched [128, 2, 512] across psum-bank pairs; column sums are
    built as a DVE pair/quad/oct tree and hit the PE as one ones-matmul per
    8 key chunks with a full 128-wide stationary, so every psum row carries
    the colsum and the normalize is reciprocal+multiply on the DVE -- no
    partition broadcast.
  * scores/o matmuls and the causal mask are trimmed to live columns of
    diagonal tiles.
  * out-proj (phase C) rides behind the attention head loop one query chunk
    back on its own psum banks; its drains run on ACT; output is bf16.
"""

import math

import numpy as np
import ml_dtypes

import concourse.bass as bass
import concourse.mybir as mybir
import concourse.tile as tile
from concourse import bacc
from concourse.bass_utils import run_bass_kernel_spmd

B, T, D = 2, 2048, 2048
H, HD = 16, 128
G = 4              # head-groups (cores per batch)
GH = H // G        # heads per core
GF = GH * HD       # projected features per core (512)
NCORES = 8
P = 128            # partitions
TQ = 512           # tq chunk width (psum bank / fp32 moving max)
F32 = mybir.dt.float32
BF16 = mybir.dt.bfloat16

DK = D // P        # 16 contraction chunks for projections
NTQ = T // TQ      # 4 query chunks
NTK = T // P       # 16 key chunks


def _body(tc, xT, xgc, wqT, wkT, woT, bqT, bkT, out):
    nc = tc.nc
    sc_exp = 1.0 / math.sqrt(HD)

    with (
        tc.tile_pool(name="const", bufs=1) as const,
        tc.tile_pool(name="res1", bufs=1) as res1,
        tc.tile_pool(name="res2", bufs=1) as res2,
        tc.tile_pool(name="wt", bufs=4) as wtp,
        tc.tile_pool(name="pr", bufs=6) as prp,
        tc.tile_pool(name="rb", bufs=4) as rbp,
        tc.tile_pool(name="ct", bufs=4) as ctp,
        tc.tile_pool(name="psO", bufs=1, space="PSUM") as psO,
        tc.tile_pool(name="psSum", bufs=1, space="PSUM") as psSum,
    ):
        qT_sb = res1.tile([P, GH, T], BF16)   # q, transposed per head
        kT_sb = res1.tile([P, GH, T], BF16)

        bq_sb = const.tile([HD, GH], F32)
        nc.sync.dma_start(out=bq_sb, in_=bqT)
        bk_sb = const.tile([HD, GH], F32)
        nc.sync.dma_start(out=bk_sb, in_=bkT)

        xg_sb = res2.tile([P, NTK, GF], BF16)  # (x*c)[:, gsl] by tk chunk
        oT_sb = res2.tile([P, GH, T], BF16)    # attention out, transposed
        wo_sb = res2.tile([P, GH, D], BF16)    # Wo[:, gsl].T chunked
        ones_sb = res2.tile([P, P], BF16)
        nc.vector.memset(ones_sb, 1.0)

        def emit_head(j, h, psS_pool, afill=None, nfill=0):
            """One attention head for query chunk j.  PE work that depends
            on an exp is lagged one s-pair; when `afill` is given, phase-A
            matmul groups are pulled in after each exp as extra cover."""
            qsl = slice(j * TQ, (j + 1) * TQ)
            nkk = (j + 1) * (TQ // P)
            npair = nkk // 2
            ncs = (j + 1) // 2 + (j + 1) % 2
            ps_o = psO.tile([P, TQ], F32, name="ps_o")
            ps_sum = psSum.tile([P, TQ], F32, name="ps_sum")
            wps = []
            quads = []
            ics = 0
            deferred = []
            for ip in range(npair):
                ps2 = psS_pool.tile([P, 2, TQ], F32, name="ps_s", tag="ps_s")
                for jj in (0, 1):
                    i = 2 * ip + jj
                    g = i - (TQ // P) * j
                    lo = g * P if g > 0 else 0
                    nc.tensor.matmul(
                        ps2[:, jj, lo:],
                        kT_sb[:, h, i * P : (i + 1) * P],
                        qT_sb[:, h, j * TQ + lo : (j + 1) * TQ],
                        start=True,
                        stop=True,
                    )
                wt2 = wtp.tile([P, 2, TQ], BF16, name="wt")
                nc.scalar.activation(
                    wt2, ps2, mybir.ActivationFunctionType.Exp, scale=sc_exp,
                )
                for jj in (0, 1):
                    i = 2 * ip + jj
                    g = i - (TQ // P) * j
                    if g >= 0:  # zero cols left of + on the diagonal
                        nc.gpsimd.affine_select(
                            out=wt2[:, jj, 0 : (g + 1) * P],
                            in_=wt2[:, jj, 0 : (g + 1) * P],
                            pattern=[[1, (g + 1) * P]],
                            compare_op=mybir.AluOpType.is_ge,
                            fill=0.0,
                            base=-(P * g),
                            channel_multiplier=-1,
                        )
                for fn in deferred:
                    fn()
                deferred = []
                if afill is not None:
                    for _ in range(nfill):
                        grp = next(afill, None)
                        if grp is not None:
                            grp()
                wp = prp.tile([P, TQ], BF16, name="wp")
                nc.vector.tensor_add(wp, wt2[:, 0, :], wt2[:, 1, :])
                wps.append(wp)

                def _mk_o(ipc=ip, wt2c=wt2):
                    def emit():
                        for jj in (0, 1):
                            i = 2 * ipc + jj
                            g = i - (TQ // P) * j
                            lo = g * P if g > 0 else 0
                            nc.tensor.matmul(
                                ps_o[:, lo:],
                                xg_sb[:, i, h * HD : (h + 1) * HD],
                                wt2c[:, jj, lo:],
                                start=(i == 0),
                                stop=(i == nkk - 1),
                            )
                    return emit

                deferred.append(_mk_o())
                if ip % 2 == 1:
                    wq4 = prp.tile([P, TQ], BF16, name="wq4")
                    nc.vector.tensor_add(wq4, wps[-2], wps[-1])
                    quads.append(wq4)
                    emit_cs = None
                    if len(quads) == 2:
                        oc = prp.tile([P, TQ], BF16, name="oc")
                        nc.vector.tensor_add(oc, quads[0], quads[1])
                        quads = []
                        emit_cs = oc
                    elif ip == npair - 1:
                        emit_cs = quads[0]
                        quads = []
                    if emit_cs is not None:
                        def _mk_cs(icsc=ics, rhs=emit_cs):
                            def emit():
                                nc.tensor.matmul(
                                    ps_sum, ones_sb, rhs,
                                    start=(icsc == 0),
                                    stop=(icsc == ncs - 1),
                                )
                            return emit

                        deferred.append(_mk_cs())
                        ics += 1
            for fn in deferred:
                fn()
            # normalize immediately: recip of the row-broadcast colsums,
            # then scale ps_o on the DVE.  The PE moves on with no
            # dependency on this chain.
            rb = rbp.tile([P, TQ], F32, name="rb")
            nc.vector.reciprocal_approx_fast(out=rb, in_=ps_sum)
            nc.vector.tensor_mul(oT_sb[:, h, qsl], ps_o, rb)

        # --- phase A: q/k projections (bf16, dk-inner), with the j=0
        # attention chunk woven between its last three x-chunks ---
        with (
            tc.tile_pool(name="wqk", bufs=1) as wqk,
            tc.tile_pool(name="xt", bufs=2) as xtp,
            tc.tile_pool(name="psA", bufs=2, space="PSUM") as psA,
            tc.tile_pool(name="psSw", bufs=2, space="PSUM") as psSw,
        ):
            wq_sb = wqk.tile([P, DK, GF], BF16)
            wk_sb = wqk.tile([P, DK, GF], BF16)
            # chunk the first DMAs so the first matmul (dk=0) starts as
            # soon as the first slices land; wk is only needed after the
            # whole q pass, so its chunks go last
            xt0 = xtp.tile([P, DK, TQ], BF16, name="xt")
            slices = [slice(d, d + 1) for d in range(4)] + [
                slice(4 + 2 * c, 6 + 2 * c) for c in range(6)
            ]
            for dsl in slices:
                nc.sync.dma_start(out=wq_sb[:, dsl, :], in_=wqT[:, dsl, :])
                nc.sync.dma_start(out=xt0[:, dsl, :], in_=xT[0][:, dsl, :])
            # gpsimd library (affine_select) fetch rides behind the
            # first-matmul-critical chunks; first use is much later
            from concourse import library_config
            nc.gpsimd.load_library(library_config.attn)
            for dsl in slices:
                nc.sync.dma_start(out=wk_sb[:, dsl, :], in_=wkT[:, dsl, :])
            nc.sync.dma_start(out=xg_sb, in_=xgc)
            nc.sync.dma_start(out=wo_sb, in_=woT)

            def _mk_agroup(xt, tsl, w_sb, dstT, bias, h, on_dve=False):
                def emit():
                    ps = psA.tile([P, TQ], F32, name="psA_t")
                    for dk in range(DK):
                        nc.tensor.matmul(
                            ps,
                            w_sb[:, dk, h * HD : (h + 1) * HD],
                            xt[:, dk, :],
                            start=(dk == 0),
                            stop=(dk == DK - 1),
                        )
                    if on_dve:
                        # interleaved groups drain on the DVE: the ACT queue
                        # is busy with the j=0 exps there
                        nc.vector.tensor_scalar(
                            dstT[:, h, tsl], ps, bias[:, h : h + 1], None,
                            mybir.AluOpType.add,
                        )
                    else:
                        nc.scalar.activation(
                            dstT[:, h, tsl],
                            ps,
                            mybir.ActivationFunctionType.Identity,
                            bias=bias[:, h : h + 1],
                        )
                return emit

            agroups = []
            for tci in range(NTQ):
                tsl = slice(tci * TQ, (tci + 1) * TQ)
                if tci == 0:
                    xt = xt0
                else:
                    xt = xtp.tile([P, DK, TQ], BF16, name="xt")
                    nc.sync.dma_start(out=xt, in_=xT[tci])
                for w_sb, dstT, bias in (
                    (wq_sb, qT_sb, bq_sb),
                    (wk_sb, kT_sb, bk_sb),
                ):
                    for h in range(GH):
                        agroups.append(
                            _mk_agroup(xt, tsl, w_sb, dstT, bias, h,
                                       on_dve=(tci > 0))
                        )

            for grp in agroups[: 2 * GH]:  # all of tci=0
                grp()
            afill = iter(agroups[2 * GH :])  # 24 groups left
            for h in range(GH):
                emit_head(0, h, psSw, afill=afill, nfill=3)
            for grp in afill:
                grp()

        # --- query chunks 1..3 with out-proj riding one chunk behind ---
        with (
            tc.tile_pool(name="psS", bufs=2, space="PSUM") as psS,
            tc.tile_pool(name="psC", bufs=2, space="PSUM") as psC,
        ):
            for j in range(1, NTQ):
                for h in range(GH):
                    emit_head(j, h, psS)
                    _emit_outproj(nc, psC, ctp, wo_sb, oT_sb, out,
                                  j - 1, (4 * h, 4 * h + 4))
            _emit_outproj(nc, psC, ctp, wo_sb, oT_sb, out, NTQ - 1, (0, DK))


def _emit_outproj(nc, psC, ctp, wo_sb, oT_sb, out, j, dk_range):
    qsl = slice(j * TQ, (j + 1) * TQ)
    for dk in range(*dk_range):
        ps = psC.tile([P, TQ], F32, name="psC_t")
        for m in range(GH):
            nc.tensor.matmul(
                ps,
                wo_sb[:, m, dk * P : (dk + 1) * P],
                oT_sb[:, m, qsl],
                start=(m == 0),
                stop=(m == GH - 1),
            )
        ct = ctp.tile([P, TQ], BF16, name="ct")
        nc.scalar.copy(ct, ps)
        nc.sync.dma_start(out=out[:, dk, qsl], in_=ct)


def build_program():
    nc = bacc.Bacc(
        "TRN2", target_bir_lowering=False, debug=False, num_devices=NCORES
    )
    xT = nc.dram_tensor("xT", [NTQ, P, DK, TQ], BF16, kind="ExternalInput").ap()
    xgc = nc.dram_tensor("xgc", [P, NTK, GF], BF16, kind="ExternalInput").ap()
    wqT = nc.dram_tensor("wqT", [P, DK, GF], BF16, kind="ExternalInput").ap()
    wkT = nc.dram_tensor("wkT", [P, DK, GF], BF16, kind="ExternalInput").ap()
    woT = nc.dram_tensor("woT", [P, GH, D], BF16, kind="ExternalInput").ap()
    bqT = nc.dram_tensor("bqT", [HD, GH], F32, kind="ExternalInput").ap()
    bkT = nc.dram_tensor("bkT", [HD, GH], F32, kind="ExternalInput").ap()
    out = nc.dram_tensor("out", [P, DK, T], BF16, kind="ExternalOutput").ap()

    with tile.TileContext(nc) as tc:
        _body(tc, xT, xgc, wqT, wkT, woT, bqT, bkT, out)
    nc.compile()
    return nc


_NC_CACHE = None
LAST_RESULT = None
TRACE = False


def kernel(x, Wq, bq, Wk, bk, Wvq, bvq, v_keys, v_embed, Wo, bo):
    global _NC_CACHE, LAST_RESULT
    x = np.asarray(x, np.float32)
    Wq = np.asarray(Wq, np.float32)
    bq = np.asarray(bq, np.float32)
    Wk = np.asarray(Wk, np.float32)
    bk = np.asarray(bk, np.float32)
    v_embed = np.asarray(v_embed, np.float32)
    Wo = np.asarray(Wo, np.float32)
    bo = np.asarray(bo, np.float32)

    bf = ml_dtypes.bfloat16
    c = 2.0 * v_embed[:G].sum(axis=0)

    in_maps = []
    for core in range(NCORES):
        b, g = divmod(core, G)
        gsl = slice(g * GF, (g + 1) * GF)
        # x[b].T chunked [tci][p][dk][t]: elem = x[b][tci*TQ+t, dk*P+p]
        xTd = np.ascontiguousarray(
            x[b].reshape(NTQ, TQ, DK, P).transpose(0, 3, 2, 1)
        ).astype(bf)
        # (x*c) slice chunked [p][i][f]: elem = (x*c)[b][i*P+p, gsl.start+f]
        xgcd = np.ascontiguousarray(
            (x[b][:, gsl] * c[gsl]).reshape(NTK, P, GF).transpose(1, 0, 2)
        ).astype(bf)
        # Wq[gsl].T chunked [p][dk][f]: elem = Wq[gsl][f, dk*P+p]
        wqTd = np.ascontiguousarray(
            Wq[gsl, :].T.reshape(DK, P, GF).transpose(1, 0, 2)
        ).astype(bf)
        wkTd = np.ascontiguousarray(
            Wk[gsl, :].T.reshape(DK, P, GF).transpose(1, 0, 2)
        ).astype(bf)
        # Wo[:, gsl].T chunked [p][m][d]: elem = Wo[d, gsl.start + m*P+p]
        woTd = np.ascontiguousarray(
            Wo[:, gsl].T.reshape(GH, P, D).transpose(1, 0, 2)
        ).astype(bf)
        in_maps.append(
            {
                "xT": xTd,
                "xgc": xgcd,
                "wqT": wqTd,
                "wkT": wkTd,
                "woT": woTd,
                "bqT": np.ascontiguousarray(bq[gsl].reshape(GH, HD).T),
                "bkT": np.ascontiguousarray(bk[gsl].reshape(GH, HD).T),
            }
        )

    if _NC_CACHE is None:
        _NC_CACHE = build_program()
    res = run_bass_kernel_spmd(
        _NC_CACHE, in_maps, list(range(NCORES)), trace=TRACE
    )
    LAST_RESULT = res

    out = np.zeros((B, T, D), np.float32)
    for core in range(NCORES):
        b = core // G
        # out dram [p][dk][t]: elem = outT[dk*P+p, t] -> out[b][t, dk*P+p]
        o = res.results[core]["out"].astype(np.float32)
        out[b] += o.transpose(2, 1, 0).reshape(T, D)
    out += bo[None, None, :]
    return out


if __name__ == "__main__":
    nc = build_program()
    print("built ok")


# revision 16
# speedup vs baseline: 1.0472x; 1.0109x over previous
"""Trainium2 Bass kernel for nn_DMHA_3255585210402 (retrieval_knn DMHA).

Key algebraic fact: TOPK == NVK == 4, so jax.lax.top_k over the size-4 v_keys
axis selects *all* entries; the gather+sum over (DVH, TOPK) therefore reduces
to a constant vector c = 2 * v_embed[0:4].sum(0), and the whole
compute_value_states branch collapses to  v = x * c.

So the module is a causal MHA layer (B=2, H=16, T=2048, HD=128, D=2048) with
elementwise-scaled V.  Sharding: 8 cores = 2 batches x 4 head-groups.

All matmuls run in bf16 (same 1 cycle/row as fp32r on the PE but FWL weight
loads kick in, and ACT/DVE/SBUF byte traffic halves).  fp8 was measured and
rejected: softmax-logit noise from fp8 q/k lands directly on the output
(rel err 4.5e-2 > 2e-2 gate) because V is random -- no averaging rescue.

Schedule highlights:
  * phase A (q/k projections) streams x dk-inner, one psum group per
    (weight, head); drains pipeline on ACT.  The first query chunk of
    attention is woven between the last three x-chunks of phase A so the
    exp latency of those thin heads hides behind queued projection matmuls.
  * c is folded into the V operand on the host (xgc = x * c); V is never
    materialized.
  * exps are batched [128, 2, 512] across psum-bank pairs; column sums are
    built as a DVE pair/quad/oct tree and hit the PE as one ones-matmul per
    8 key chunks with a full 128-wide stationary, so every psum row carries
    the colsum and the normalize is reciprocal+multiply on the DVE -- no
    partition broadcast.
  * scores/o matmuls and the causal mask are trimmed to live columns of
    diagonal tiles.
  * out-proj (phase C) rides behind the attention head loop one query chunk
    back on its own psum banks; its drains run on ACT; output is bf16.
"""

import math

import numpy as np
import ml_dtypes

import concourse.bass as bass
import concourse.mybir as mybir
import concourse.tile as tile
from concourse import bacc
from concourse.bass_utils import run_bass_kernel_spmd

B, T, D = 2, 2048, 2048
H, HD = 16, 128
G = 4              # head-groups (cores per batch)
GH = H // G        # heads per core
GF = GH * HD       # projected features per core (512)
NCORES = 8
P = 128            # partitions
TQ = 512           # tq chunk width (psum bank / fp32 moving max)
F32 = mybir.dt.float32
BF16 = mybir.dt.bfloat16

DK = D // P        # 16 contraction chunks for projections
NTQ = T // TQ      # 4 query chunks
NTK = T // P       # 16 key chunks


def _body(tc, xT, xgc, wqT, wkT, woT, bqT, bkT, out):
    nc = tc.nc
    sc_exp = 1.0 / math.sqrt(HD)

    with (
        tc.tile_pool(name="const", bufs=1) as const,
        tc.tile_pool(name="res1", bufs=1) as res1,
        tc.tile_pool(name="res2", bufs=1) as res2,
        tc.tile_pool(name="wt", bufs=4) as wtp,
        tc.tile_pool(name="pr", bufs=6) as prp,
        tc.tile_pool(name="rb", bufs=4) as rbp,
        tc.tile_pool(name="ct", bufs=4) as ctp,
        tc.tile_pool(name="psO", bufs=1, space="PSUM") as psO,
        tc.tile_pool(name="psSum", bufs=1, space="PSUM") as psSum,
    ):
        qT_sb = res1.tile([P, GH, T], BF16)   # q, transposed per head
        kT_sb = res1.tile([P, GH, T], BF16)

        bq_sb = const.tile([HD, GH], F32)
        nc.sync.dma_start(out=bq_sb, in_=bqT)
        bk_sb = const.tile([HD, GH], F32)
        nc.sync.dma_start(out=bk_sb, in_=bkT)

        xg_sb = res2.tile([P, NTK, GF], BF16)  # (x*c)[:, gsl] by tk chunk
        oT_sb = res2.tile([P, GH, T], BF16)    # attention out, transposed
        wo_sb = res2.tile([P, GH, D], BF16)    # Wo[:, gsl].T chunked
        ones_sb = res2.tile([P, P], BF16)
        nc.vector.memset(ones_sb, 1.0)

        def emit_head(j, h, psS_pool, afill=None, nfill=0):
            """One attention head for query chunk j.  PE work that depends
            on an exp is lagged one s-pair; when `afill` is given, phase-A
            matmul groups are pulled in after each exp as extra cover."""
            qsl = slice(j * TQ, (j + 1) * TQ)
            nkk = (j + 1) * (TQ // P)
            npair = nkk // 2
            ncs = (j + 1) // 2 + (j + 1) % 2
            ps_o = psO.tile([P, TQ], F32, name="ps_o")
            ps_sum = psSum.tile([P, TQ], F32, name="ps_sum")
            wps = []
            quads = []
            ics = 0
            deferred = []  # batches of PE work, emitted two s-pairs late so
            # the first ps_o/ps_sum writes land after the previous head's
            # normalize chain has released the single-buffered banks
            batch = []
            for ip in range(npair):
                ps2 = psS_pool.tile([P, 2, TQ], F32, name="ps_s", tag="ps_s")
                for jj in (0, 1):
                    i = 2 * ip + jj
                    g = i - (TQ // P) * j
                    lo = g * P if g > 0 else 0
                    nc.tensor.matmul(
                        ps2[:, jj, lo:],
                        kT_sb[:, h, i * P : (i + 1) * P],
                        qT_sb[:, h, j * TQ + lo : (j + 1) * TQ],
                        start=True,
                        stop=True,
                    )
                wt2 = wtp.tile([P, 2, TQ], BF16, name="wt")
                nc.scalar.activation(
                    wt2, ps2, mybir.ActivationFunctionType.Exp, scale=sc_exp,
                )
                for jj in (0, 1):
                    i = 2 * ip + jj
                    g = i - (TQ // P) * j
                    if g >= 0:  # zero cols left of + on the diagonal
                        nc.gpsimd.affine_select(
                            out=wt2[:, jj, 0 : (g + 1) * P],
                            in_=wt2[:, jj, 0 : (g + 1) * P],
                            pattern=[[1, (g + 1) * P]],
                            compare_op=mybir.AluOpType.is_ge,
                            fill=0.0,
                            base=-(P * g),
                            channel_multiplier=-1,
                        )
                while len(deferred) >= 2:
                    for fn in deferred.pop(0):
                        fn()
                if afill is not None:
                    for _ in range(nfill):
                        grp = next(afill, None)
                        if grp is not None:
                            grp()
                wp = prp.tile([P, TQ], BF16, name="wp")
                nc.vector.tensor_add(wp, wt2[:, 0, :], wt2[:, 1, :])
                wps.append(wp)

                def _mk_o(ipc=ip, wt2c=wt2):
                    def emit():
                        for jj in (0, 1):
                            i = 2 * ipc + jj
                            g = i - (TQ // P) * j
                            lo = g * P if g > 0 else 0
                            nc.tensor.matmul(
                                ps_o[:, lo:],
                                xg_sb[:, i, h * HD : (h + 1) * HD],
                                wt2c[:, jj, lo:],
                                start=(i == 0),
                                stop=(i == nkk - 1),
                            )
                    return emit

                batch = [_mk_o()]
                if ip % 2 == 1:
                    wq4 = prp.tile([P, TQ], BF16, name="wq4")
                    nc.vector.tensor_add(wq4, wps[-2], wps[-1])
                    quads.append(wq4)
                    emit_cs = None
                    if len(quads) == 2:
                        oc = prp.tile([P, TQ], BF16, name="oc")
                        nc.vector.tensor_add(oc, quads[0], quads[1])
                        quads = []
                        emit_cs = oc
                    elif ip == npair - 1:
                        emit_cs = quads[0]
                        quads = []
                    if emit_cs is not None:
                        def _mk_cs(icsc=ics, rhs=emit_cs):
                            def emit():
                                nc.tensor.matmul(
                                    ps_sum, ones_sb, rhs,
                                    start=(icsc == 0),
                                    stop=(icsc == ncs - 1),
                                )
                            return emit

                        batch.append(_mk_cs())
                        ics += 1
                deferred.append(batch)
            for b in deferred:
                for fn in b:
                    fn()
            # normalize immediately: recip of the row-broadcast colsums,
            # then scale ps_o on the DVE.  The PE moves on with no
            # dependency on this chain.
            rb = rbp.tile([P, TQ], F32, name="rb")
            nc.vector.reciprocal_approx_fast(out=rb, in_=ps_sum)
            nc.vector.tensor_mul(oT_sb[:, h, qsl], ps_o, rb)

        # --- phase A: q/k projections (bf16, dk-inner), with the j=0
        # attention chunk woven between its last three x-chunks ---
        with (
            tc.tile_pool(name="wqk", bufs=1) as wqk,
            tc.tile_pool(name="xt", bufs=2) as xtp,
            tc.tile_pool(name="psA", bufs=2, space="PSUM") as psA,
            tc.tile_pool(name="psSw", bufs=2, space="PSUM") as psSw,
        ):
            wq_sb = wqk.tile([P, DK, GF], BF16)
            wk_sb = wqk.tile([P, DK, GF], BF16)
            # chunk the first DMAs so the first matmul (dk=0) starts as
            # soon as the first slices land; wk is only needed after the
            # whole q pass, so its chunks go last
            xt0 = xtp.tile([P, DK, TQ], BF16, name="xt")
            slices = [slice(d, d + 1) for d in range(4)] + [
                slice(4 + 2 * c, 6 + 2 * c) for c in range(6)
            ]
            for dsl in slices:
                nc.sync.dma_start(out=wq_sb[:, dsl, :], in_=wqT[:, dsl, :])
                nc.sync.dma_start(out=xt0[:, dsl, :], in_=xT[0][:, dsl, :])
            # gpsimd library (affine_select) fetch rides behind the
            # first-matmul-critical chunks; first use is much later
            from concourse import library_config
            nc.gpsimd.load_library(library_config.attn)
            for dsl in slices:
                nc.sync.dma_start(out=wk_sb[:, dsl, :], in_=wkT[:, dsl, :])
            nc.sync.dma_start(out=xg_sb, in_=xgc)
            nc.sync.dma_start(out=wo_sb, in_=woT)

            def _mk_agroup(xt, tsl, w_sb, dstT, bias, h, on_dve=False):
                def emit():
                    ps = psA.tile([P, TQ], F32, name="psA_t")
                    for dk in range(DK):
                        nc.tensor.matmul(
                            ps,
                            w_sb[:, dk, h * HD : (h + 1) * HD],
                            xt[:, dk, :],
                            start=(dk == 0),
                            stop=(dk == DK - 1),
                        )
                    if on_dve:
                        # interleaved groups drain on the DVE: the ACT queue
                        # is busy with the j=0 exps there
                        nc.vector.tensor_scalar(
                            dstT[:, h, tsl], ps, bias[:, h : h + 1], None,
                            mybir.AluOpType.add,
                        )
                    else:
                        nc.scalar.activation(
                            dstT[:, h, tsl],
                            ps,
                            mybir.ActivationFunctionType.Identity,
                            bias=bias[:, h : h + 1],
                        )
                return emit

            agroups = []
            for tci in range(NTQ):
                tsl = slice(tci * TQ, (tci + 1) * TQ)
                if tci == 0:
                    xt = xt0
                else:
                    xt = xtp.tile([P, DK, TQ], BF16, name="xt")
                    nc.sync.dma_start(out=xt, in_=xT[tci])
                for w_sb, dstT, bias in (
                    (wq_sb, qT_sb, bq_sb),
                    (wk_sb, kT_sb, bk_sb),
                ):
                    for h in range(GH):
                        agroups.append(
                            _mk_agroup(xt, tsl, w_sb, dstT, bias, h,
                                       on_dve=(tci > 0))
                        )

            for grp in agroups[: 2 * GH]:  # all of tci=0
                grp()
            afill = iter(agroups[2 * GH :])  # 24 groups left
            for h in range(GH):
                emit_head(0, h, psSw, afill=afill, nfill=3)
            for grp in afill:
                grp()

        # --- query chunks 1..3 with out-proj riding one chunk behind ---
        with (
            tc.tile_pool(name="psS", bufs=2, space="PSUM") as psS,
            tc.tile_pool(name="psC", bufs=2, space="PSUM") as psC,
        ):
            for j in range(1, NTQ):
                for h in range(GH):
                    emit_head(j, h, psS)
                    _emit_outproj(nc, psC, ctp, wo_sb, oT_sb, out,
                                  j - 1, (4 * h, 4 * h + 4))
            _emit_outproj(nc, psC, ctp, wo_sb, oT_sb, out, NTQ - 1, (0, DK))


def _emit_outproj(nc, psC, ctp, wo_sb, oT_sb, out, j, dk_range):
    qsl = slice(j * TQ, (j + 1) * TQ)
    for dk in range(*dk_range):
        ps = psC.tile([P, TQ], F32, name="psC_t")
        for m in range(GH):
            nc.tensor.matmul(
                ps,
                wo_sb[:, m, dk * P : (dk + 1) * P],
                oT_sb[:, m, qsl],
                start=(m == 0),
                stop=(m == GH - 1),
            )
        ct = ctp.tile([P, TQ], BF16, name="ct")
        nc.scalar.copy(ct, ps)
        nc.sync.dma_start(out=out[:, dk, qsl], in_=ct)


def build_program():
    nc = bacc.Bacc(
        "TRN2", target_bir_lowering=False, debug=False, num_devices=NCORES
    )
    xT = nc.dram_tensor("xT", [NTQ, P, DK, TQ], BF16, kind="ExternalInput").ap()
    xgc = nc.dram_tensor("xgc", [P, NTK, GF], BF16, kind="ExternalInput").ap()
    wqT = nc.dram_tensor("wqT", [P, DK, GF], BF16, kind="ExternalInput").ap()
    wkT = nc.dram_tensor("wkT", [P, DK, GF], BF16, kind="ExternalInput").ap()
    woT = nc.dram_tensor("woT", [P, GH, D], BF16, kind="ExternalInput").ap()
    bqT = nc.dram_tensor("bqT", [HD, GH], F32, kind="ExternalInput").ap()
    bkT = nc.dram_tensor("bkT", [HD, GH], F32, kind="ExternalInput").ap()
    out = nc.dram_tensor("out", [P, DK, T], BF16, kind="ExternalOutput").ap()

    with tile.TileContext(nc) as tc:
        _body(tc, xT, xgc, wqT, wkT, woT, bqT, bkT, out)
    nc.compile()
    return nc


_NC_CACHE = None
LAST_RESULT = None
TRACE = False


def kernel(x, Wq, bq, Wk, bk, Wvq, bvq, v_keys, v_embed, Wo, bo):
    global _NC_CACHE, LAST_RESULT
    x = np.asarray(x, np.float32)
    Wq = np.asarray(Wq, np.float32)
    bq = np.asarray(bq, np.float32)
    Wk = np.asarray(Wk, np.float32)
    bk = np.asarray(bk, np.float32)
    v_embed = np.asarray(v_embed, np.float32)
    Wo = np.asarray(Wo, np.float32)
    bo = np.asarray(bo, np.float32)

    bf = ml_dtypes.bfloat16
    c = 2.0 * v_embed[:G].sum(axis=0)

    in_maps = []
    for core in range(NCORES):
        b, g = divmod(core, G)
        gsl = slice(g * GF, (g + 1) * GF)
        # x[b].T chunked [tci][p][dk][t]: elem = x[b][tci*TQ+t, dk*P+p]
        xTd = np.ascontiguousarray(
            x[b].reshape(NTQ, TQ, DK, P).transpose(0, 3, 2, 1)
        ).astype(bf)
        # (x*c) slice chunked [p][i][f]: elem = (x*c)[b][i*P+p, gsl.start+f]
        xgcd = np.ascontiguousarray(
            (x[b][:, gsl] * c[gsl]).reshape(NTK, P, GF).transpose(1, 0, 2)
        ).astype(bf)
        # Wq[gsl].T chunked [p][dk][f]: elem = Wq[gsl][f, dk*P+p]
        wqTd = np.ascontiguousarray(
            Wq[gsl, :].T.reshape(DK, P, GF).transpose(1, 0, 2)
        ).astype(bf)
        wkTd = np.ascontiguousarray(
            Wk[gsl, :].T.reshape(DK, P, GF).transpose(1, 0, 2)
        ).astype(bf)
        # Wo[:, gsl].T chunked [p][m][d]: elem = Wo[d, gsl.start + m*P+p]
        woTd = np.ascontiguousarray(
            Wo[:, gsl].T.reshape(GH, P, D).transpose(1, 0, 2)
        ).astype(bf)
        in_maps.append(
            {
                "xT": xTd,
                "xgc": xgcd,
                "wqT": wqTd,
                "wkT": wkTd,
                "woT": woTd,
                "bqT": np.ascontiguousarray(bq[gsl].reshape(GH, HD).T),
                "bkT": np.ascontiguousarray(bk[gsl].reshape(GH, HD).T),
            }
        )

    if _NC_CACHE is None:
        _NC_CACHE = build_program()
    res = run_bass_kernel_spmd(
        _NC_CACHE, in_maps, list(range(NCORES)), trace=TRACE
    )
    LAST_RESULT = res

    out = np.zeros((B, T, D), np.float32)
    for core in range(NCORES):
        b = core // G
        # out dram [p][dk][t]: elem = outT[dk*P+p, t] -> out[b][t, dk*P+p]
        o = res.results[core]["out"].astype(np.float32)
        out[b] += o.transpose(2, 1, 0).reshape(T, D)
    out += bo[None, None, :]
    return out


if __name__ == "__main__":
    nc = build_program()
    print("built ok")
